# revision 2
# baseline (speedup 1.0000x reference)
"""Peephole-LSTM Trainium2 kernel (Bass/Tile), batch-parallel over 8 cores.

Problem: B=32, T=2048, F=128, H=256.
  xw = x @ Wx.T + b  (precomputed on device, bf16, SBUF-resident)
  per step: gates = xw_t + h @ Wh.T (+ peepholes), fp32 state, bf16 matmuls.

Layouts (per core, BC=4 batch rows):
  xT DRAM (128, T*4)      x^T, col = t*4+b, bf16
  xw SBUF (128, T*32)     col = s*32 + m*4 + b, m = gate tile (i,i,f,f,o,o,g,g)
  state tiles (128, 8)    col = half*4 + b   (half = gate-dim half)
  out_h/out_c DRAM (128, T*8)  col = s*8 + half*4 + b, fp32
"""

import numpy as np
import ml_dtypes

import concourse.bass as bass
import concourse.bacc as bacc
from concourse.tile import add_dep_helper
import concourse.mybir as mybir
import concourse.tile as tile
from concourse.bass_utils import run_bass_kernel_spmd

H = 256
F = 128
B = 32
T = 2048
NCORES = 8
BC = B // NCORES  # 4
GATE = 4 * H  # 1024; gate order follows the reference split: i, f, o, g
CH = 128  # steps per output-staging block

BF16 = mybir.dt.bfloat16
F32 = mybir.dt.float32
AF = mybir.ActivationFunctionType
OP = mybir.AluOpType

_prog_cache = {}
DEBUG = False
import os
ABLATE = set(os.environ.get("LSTM_KERNEL_ABLATE", "").split(",")) - {""}


def _build_program(t_steps):
    nc = bacc.Bacc("TRN2", target_bir_lowering=False, debug=False)
    tb = t_steps * BC

    xT = nc.dram_tensor("xT", [F, tb], BF16, kind="ExternalInput")
    w1 = nc.dram_tensor("W1", [4, 128, GATE], BF16, kind="ExternalInput")
    wco = nc.dram_tensor("WcoT", [2, 128, H], BF16, kind="ExternalInput")
    wx = nc.dram_tensor("WxT", [F, GATE], BF16, kind="ExternalInput")
    bias = nc.dram_tensor("bias8", [F, 8], F32, kind="ExternalInput")
    ident = nc.dram_tensor("ident", [128, 128], BF16, kind="ExternalInput")
    out_h = nc.dram_tensor("out_h", [128, t_steps * 8], BF16, kind="ExternalOutput")
    out_c = nc.dram_tensor("out_c", [128, t_steps * 8], BF16, kind="ExternalOutput")

    csz = min(512, tb)  # phase-1 moving chunk size (cols = steps*4)
    n_chunks = tb // csz
    spc = csz // 4  # steps per chunk
    ch = min(CH, t_steps)
    if DEBUG:
        xw_dbg = nc.dram_tensor("xw_dbg", [128, t_steps * 32], BF16, kind="ExternalOutput")
        pre_dbg = nc.dram_tensor("pre_dbg", [128, t_steps * 32], F32, kind="ExternalOutput")

    with tile.TileContext(nc) as tc:
        with (
            tc.tile_pool(name="const", bufs=1) as cpool,
            tc.tile_pool(name="xwp", bufs=1) as xwpool,
            tc.tile_pool(name="state", bufs=1) as spool,
            tc.tile_pool(name="xin", bufs=3) as xpool,
            tc.tile_pool(name="gat", bufs=3) as gpool,
            tc.tile_pool(name="stg", bufs=3) as stpool,
        ):
            # ---- constants ----
            w1_sb = cpool.tile([128, 4 * GATE], BF16, tag="w1")
            for kz in range(4):
                nc.sync.dma_start(w1_sb[:, kz * GATE:(kz + 1) * GATE], w1[kz])
            wco_sb = cpool.tile([128, 2 * H], BF16, tag="wco")
            for k in range(2):
                nc.sync.dma_start(wco_sb[:, k * H:(k + 1) * H], wco[k])
            wx_sb = cpool.tile([128, GATE], BF16, tag="wx")
            nc.sync.dma_start(wx_sb[:], wx.ap())
            bias_sb = cpool.tile([128, 8], F32, tag="bias")
            nc.sync.dma_start(bias_sb[:], bias.ap())
            id_sb = cpool.tile([128, 128], BF16, tag="ident")
            nc.sync.dma_start(id_sb[:], ident.ap())

            # ---- phase 1: xw = x @ Wx.T + bias (bf16, SBUF-resident) ----
            xw_sb = xwpool.tile([128, t_steps * 32], BF16, tag="xw")
            assert tb % csz == 0
            xw3 = xw_sb[:].rearrange("p (s g) -> p s g", g=32)
            with tc.tile_pool(name="ps1", bufs=4, space=bass.MemorySpace.PSUM) as ps1:
              for n in range(n_chunks):
                  xchunk = xpool.tile([128, csz], BF16, tag="xchunk")
                  nc.sync.dma_start(xchunk[:], xT.ap()[:, n * csz:(n + 1) * csz])
                  for m in range(8):
                      ps = ps1.tile([128, csz], F32, tag="ps1")
                      nc.tensor.matmul(
                          ps[:], wx_sb[:, m * 128:(m + 1) * 128], xchunk[:],
                          start=True, stop=True,
                      )
                      src = ps[:].rearrange("p (s b) -> p s b", b=4)
                      dst = xw3[:, n * spc:(n + 1) * spc, m * 4:(m + 1) * 4]
                      nc.scalar.activation(
                          dst, src, AF.Identity, bias=bias_sb[:, m:m + 1]
                      )

            if DEBUG:
                nc.sync.dma_start(xw_dbg.ap()[:], xw_sb[:])

            # ---- phase 2: recurrence ----
            h0 = spool.tile([128, 8], BF16, tag="h0")
            c0 = spool.tile([128, 8], BF16, tag="c0")
            nc.gpsimd.memset(h0[:], 0.0)
            nc.gpsimd.memset(c0[:], 0.0)

            c_prev = c0[:]
            cmm_prev = c0[:]
            h_prev = h0[:]
            stage_h = stage_c = None
            ps2cm = tc.tile_pool(name="ps2", bufs=3, space=bass.MemorySpace.PSUM)
            ps2 = ps2cm.__enter__()
            for s in range(t_steps):
                blk, off = divmod(s, ch)
                if off == 0:
                    stage_h = stpool.tile([128, ch * 8], BF16, tag="stage_h")
                    stage_c = stpool.tile([128, ch * 8], BF16, tag="stage_c")

                ps = ps2.tile([128, 24], F32, tag="ps2")
                ps_o = ps2.tile([128, 8], F32, tag="ps2o")
                # xw injection (identity matmuls) — must execute first: start=True
                ab_mm = "mm" in ABLATE
                mm_id = nc.tensor.matmul(
                    ps[:], id_sb[:], xw_sb[:, s * 32:s * 32 + 24],
                    start=True, stop=ab_mm,
                )
                mm_ido = nc.tensor.matmul(
                    ps_o[:], id_sb[:], xw_sb[:, s * 32 + 24:s * 32 + 32],
                    start=True, stop=ab_mm,
                )
                ps_mms, pso_mms = [], []

                def _mm(bank, out_ap, lhsT, rhs, stop=False):
                    first = mm_id if bank == 0 else mm_ido
                    lst = ps_mms if bank == 0 else pso_mms
                    mm = nc.tensor.matmul(out_ap, lhsT, rhs, start=False, stop=stop)
                    add_dep_helper(mm.ins, first.ins, reason="psum start first")
                    if stop:
                        for prev in lst:
                            add_dep_helper(mm.ins, prev.ins, reason="psum stop last")
                    lst.append(mm)
                    return mm

                # peepholes: i,f gate tiles (m=0..3), c halves (kz=2,3)
                for m in (range(0) if "mm" in ABLATE else range(4)):
                    for kc in range(2):
                        _mm(0, ps[:, m * 4:(m + 1) * 4],
                            w1_sb[:, (2 + kc) * GATE + m * 128:(2 + kc) * GATE + (m + 1) * 128],
                            cmm_prev[:, kc * 4:(kc + 1) * 4])
                # h part, i/f/g tiles (m=0..5) into ps; o tiles (m=6,7) into ps_o
                for m in (range(0) if "mm" in ABLATE else range(8)):
                    for kh in range(2):
                        w_ap = w1_sb[:, kh * GATE + m * 128:kh * GATE + (m + 1) * 128]
                        h_ap = h_prev[:, kh * 4:(kh + 1) * 4]
                        if m < 6:
                            _mm(0, ps[:, m * 4:(m + 1) * 4], w_ap, h_ap,
                                stop=(m == 5 and kh == 1))
                        else:
                            _mm(1, ps_o[:, (m - 6) * 4:(m - 5) * 4], w_ap, h_ap,
                                stop=("elem" in ABLATE and m == 7 and kh == 1))
                if DEBUG:
                    dbg = gpool.tile([128, 24], F32, tag="dbg")
                    nc.vector.tensor_copy(dbg[:], ps[:])
                    nc.sync.dma_start(pre_dbg.ap()[:, s * 32:s * 32 + 24], dbg[:])
                if "elem" in ABLATE:
                    c_slice = stage_c[:, off * 8:(off + 1) * 8]
                    h_slice = stage_h[:, off * 8:(off + 1) * 8]
                    nc.vector.tensor_copy(c_slice, ps[:, 0:8])
                    nc.vector.tensor_copy(h_slice, ps_o[:, 0:8])
                    c_prev = c_slice
                    h_prev = h_slice
                    if off == ch - 1:
                        base = blk * ch * 8
                        nc.sync.dma_start(out_h.ap()[:, base:base + ch * 8], stage_h[:])
                        nc.sync.dma_start(out_c.ap()[:, base:base + ch * 8], stage_c[:])
                    continue
                # ti, tf, gt = tanh(pre) ; i = (1+ti)/2, f = (1+tf)/2
                tg = gpool.tile([128, 24], F32, tag="tg")
                nc.scalar.activation(tg[:], ps[:, 0:24], AF.Tanh)
                # state C = 2c:  C_new = 0.5*(1+tf)*C_prev + (1+ti)*gt
                t1 = gpool.tile([128, 8], F32, tag="t1")
                nc.vector.scalar_tensor_tensor(
                    t1[:], tg[:, 8:16], 1.0, c_prev, OP.add, OP.mult)
                t2 = gpool.tile([128, 8], F32, tag="t2")
                nc.vector.scalar_tensor_tensor(
                    t2[:], tg[:, 0:8], 1.0, tg[:, 16:24], OP.add, OP.mult)
                c_slice = stage_c[:, off * 8:(off + 1) * 8]
                nc.vector.scalar_tensor_tensor(
                    c_slice, t1[:], 0.5, t2[:], OP.mult, OP.add)
                                # o peephole: Wco @ c_new into ps_o
                for m in (range(0) if "mm" in ABLATE else range(2)):
                    for k in range(2):
                        _mm(1, ps_o[:, m * 4:(m + 1) * 4],
                            wco_sb[:, k * H + m * 128:k * H + (m + 1) * 128],
                            c_slice[:, k * 4:(k + 1) * 4],
                            stop=(m == 1 and k == 1))
                tc_s = gpool.tile([128, 8], F32, tag="tc_s")
                nc.scalar.activation(tc_s[:], c_slice, AF.Tanh, scale=0.5)
                o_s = gpool.tile([128, 8], F32, tag="o_s")
                nc.scalar.activation(o_s[:], ps_o[:], AF.Sigmoid)
                h_slice = stage_h[:, off * 8:(off + 1) * 8]
                nc.vector.tensor_mul(h_slice, o_s[:], tc_s[:])

                c_prev = c_slice
                cmm_prev = c_slice
                h_prev = h_slice
                if off == ch - 1:
                    base = blk * ch * 8
                    nc.sync.dma_start(out_h.ap()[:, base:base + ch * 8], stage_h[:])
                    nc.sync.dma_start(out_c.ap()[:, base:base + ch * 8], stage_c[:])

            ps2cm.__exit__(None, None, None)

    nc.compile()
    return nc


def _pack_weights(Wx, bx, Wh, bh, Wci, bci, Wcf, bcf, Wco, bco):
    # reference gate (row) order is i, f, o, g; the device uses i, f, g, o
    bf = ml_dtypes.bfloat16
    perm = np.concatenate([
        np.arange(0, H), np.arange(H, 2 * H),
        np.arange(3 * H, 4 * H), np.arange(2 * H, 3 * H),
    ])
    Whp = Wh[perm]
    Wxp = Wx[perm]
    bp = (bx + bh)[perm]
    WhT = np.ascontiguousarray(Whp.T).copy()  # (256, 1024) [h_dim, gate]
    # sigma-trick: i,f gates computed as tanh(pre/2) -> scale i,f pre-acts by 0.5
    WhT[:, 0:2 * H] *= 0.5
    w1 = np.zeros((4, 128, GATE), np.float32)
    w1[0] = WhT[0:128]
    w1[1] = WhT[128:256]
    # c-rows: state is stored as 2c -> extra 0.5; with sigma-trick total 0.25
    ct = np.zeros((256, GATE), np.float32)
    ct[:, 0:H] = Wci.T * 0.25
    ct[:, H:2 * H] = Wcf.T * 0.25
    w1[2] = ct[0:128]
    w1[3] = ct[128:256]
    wcoT = np.ascontiguousarray(Wco.T) * 0.5  # (c_dim, o_dim); 2c state
    wco = np.stack([wcoT[0:128], wcoT[128:256]])
    bias = bp + np.concatenate([bci, bcf, np.zeros(H, np.float32), bco])
    bias[0:2 * H] *= 0.5
    Wxp = Wxp.copy()
    Wxp[0:2 * H] *= 0.5
    bias8 = np.ascontiguousarray(bias.reshape(8, 128).T, dtype=np.float32)
    return {
        "W1": w1.astype(bf),
        "WcoT": wco.astype(bf),
        "WxT": np.ascontiguousarray(Wxp.T).astype(bf),
        "bias8": bias8,
        "ident": np.eye(128, dtype=np.float32).astype(bf),
    }


def kernel(x, Wx, bx, Wh, bh, Wci, bci, Wcf, bcf, Wco, bco):
    x = np.asarray(x, np.float32)
    args = [np.asarray(a, np.float32) for a in (Wx, bx, Wh, bh, Wci, bci, Wcf, bcf, Wco, bco)]
    t_steps = x.shape[1]
    bf = ml_dtypes.bfloat16

    common = _pack_weights(*args)
    in_maps = []
    for c in range(NCORES):
        xc = x[c * BC:(c + 1) * BC]  # (4, T, 128)
        xT = np.ascontiguousarray(xc.transpose(2, 1, 0).reshape(F, t_steps * BC))
        in_maps.append({"xT": xT.astype(bf), **common})

    key = t_steps
    if key not in _prog_cache:
        _prog_cache[key] = _build_program(t_steps)
    nc = _prog_cache[key]

    res = run_bass_kernel_spmd(nc, in_maps, core_ids=list(range(NCORES)))

    hiddens = np.zeros((t_steps + 1, B, H), np.float32)
    memorys = np.zeros((t_steps + 1, B, H), np.float32)
    for c in range(NCORES):
        oh = np.asarray(res.results[c]["out_h"], np.float32).reshape(128, t_steps, 2, 4)
        oc = 0.5 * np.asarray(res.results[c]["out_c"], np.float32).reshape(128, t_steps, 2, 4)
        # [p, s, half, b] -> [s, b, half*128+p]
        hiddens[1:, c * BC:(c + 1) * BC] = oh.transpose(1, 3, 2, 0).reshape(t_steps, BC, H)
        memorys[1:, c * BC:(c + 1) * BC] = oc.transpose(1, 3, 2, 0).reshape(t_steps, BC, H)
    return hiddens, memorys

